# revision 54
# baseline (speedup 1.0000x reference)
"""Multi-head self-attention on 8 TRN2 NeuronCores.

Problem: x(4,2048,1024), Wq(8,1024,128), Wk/Wv(1024,128), Wo(1024,1024) fp32.
out = softmax(Q K^T / sqrt(128)) V -> concat heads -> @ Wo.

Sharding: (batch, query-half) across 8 cores — core c handles batch c//2,
query rows [(c%2)*1024, (c%2)*1024+1024). K/V cover the full sequence of the
batch, so each core computes them locally from its x slice; no collectives.

Numerics: scores have std ~1024 and softmax is near-one-hot, so the
x->Q/K->scores chain needs ~fp32 precision. bf16 matmuls with hi/lo split
operands ("split3": Ah*Bh + Ah*Bl + Al*Bh, fp32 PSUM accumulation) give
~5e-6 relative matmul error at 3 cycles/row (native fp32 is 4). The x and
weight splits are precomputed on the host. V/ctx/Wo paths are plain bf16.

Layouts (partition dim first):
  xT (E,S) host-transposed; K^T (O,S) = sum_e Wk[e].T-stationary @ xT[e];
  Q_h^T (O,Sq) likewise (Wq pre-scaled by 1/sqrt(O) on host);
  scores tile (128q, 2048s) = Q^T-slice-stationary @ K^T-moving, fp32 PSUM,
  bank-chunk-major so each 512-col bank finishes early;
  softmax per q-row: per-bank DVE reduce_max -> combine(negate) -> per-bank
  ACT exp(bias=-max, accum_out=den chunk) -> den sum -> 1/den -> DVE scale;
  P transposed via the DMA xbar (dma_start_transpose on the two HWDGE
  queues) — one [128,2048]->[128,16,128] instruction per q-tile, keeping
  the PE free for real matmuls (the xbar runs ~2.6us/tile-transpose
  aggregate, overlapped under the per-head matmul time);
  ctx^T (O,Sq) = V-stationary @ P^T-moving; out (Sq,E) = ctx-slices-stationary
  @ Wo-moving (natural output layout), final head's ctx interleaved with the
  out projection so the PE tail stays dense.
"""
import os
import numpy as np
import ml_dtypes

# a previous crashed run can leave the NeuronCores wedged
# (NRT_EXEC_UNIT_UNRECOVERABLE) — ask the runtime to reset on open
os.environ.setdefault("NEURON_RT_RESET_CORES", "1")

B, S, E, H, O = 4, 2048, 1024, 8, 128
SQ = S // 2          # query rows per core
NCORES = 8
ET = E // 128        # 8 e-tiles
ST = S // 128        # 16 s-tiles
QT = SQ // 128       # 8 q-tiles
NB = S // 512        # 4 score banks per q-tile
EC = E // 512        # 2 out-proj column chunks

_compiled = None     # cache so repeated kernel() calls skip rebuild


def _build():
    import concourse.bass as bass
    import concourse.mybir as mybir
    import concourse.tile as tile
    from concourse import bacc

    F32 = mybir.dt.float32
    BF16 = mybir.dt.bfloat16
    F16 = mybir.dt.float16
    PS = bass.MemorySpace.PSUM
    EXP = mybir.ActivationFunctionType.Exp

    nc = bacc.Bacc("TRN2", target_bir_lowering=False, debug=False,
                   enable_asserts=True)

    # xkv columns are pre-permuted per core so its query half is always
    # columns [0, SQ) — attention is permutation-invariant over the key axis,
    # so the same NEFF slices queries identically on every core.
    # weights come in host-rearranged so every DMA row is a contiguous
    # 2KB+ run per partition (256B descriptors gut DMA efficiency):
    #   wq[h, p, e*O+o] = Wq_scaled[h, e*128+p, o]
    #   wk[p, e*O+o] = Wk[e*128+p, o]  (same for wv)
    #   wo[p, h*E+e] = Wo[h*128+p, e]
    d_xkvh = nc.dram_tensor("xkvh", (E, S), BF16, kind="ExternalInput").ap()
    d_xkvl = nc.dram_tensor("xkvl", (E, S), BF16, kind="ExternalInput").ap()
    d_wqh = nc.dram_tensor("wqh", (H, 128, E), BF16, kind="ExternalInput").ap()
    d_wql = nc.dram_tensor("wql", (H, 128, E), BF16, kind="ExternalInput").ap()
    d_wkh = nc.dram_tensor("wkh", (128, E), BF16, kind="ExternalInput").ap()
    d_wkl = nc.dram_tensor("wkl", (128, E), BF16, kind="ExternalInput").ap()
    d_wvh = nc.dram_tensor("wvh", (128, E), BF16, kind="ExternalInput").ap()
    d_woh = nc.dram_tensor("woh", (128, H * E), BF16, kind="ExternalInput").ap()
    d_out = nc.dram_tensor("out", (SQ, E), F32, kind="ExternalOutput").ap()

    with tile.TileContext(nc) as tc:
        with (
            tc.tile_pool(name="persist", bufs=1) as persist,
            tc.tile_pool(name="tiny", bufs=24) as tiny,
        ):
            wo_sb = persist.tile([128, H, E], BF16, tag="wo")

            # fp16 (11-bit mantissa) lets the scores run in TWO passes
            # (Qh+Ql)@Kh instead of bf16 split3's three: the residual K
            # rounding of 2^-12 relative perturbs scores by ~0.3 (std 1024)
            # which measures 7.8e-3 global error vs the 2e-2 gate
            kth = persist.tile([128, S], F16, tag="kth")
            qth = persist.tile([128, H, SQ], F16, tag="qth")
            qtl = persist.tile([128, H, SQ], F16, tag="qtl")
            v_sb = persist.tile([128, ST, O], BF16, tag="v")

            # ---------------- prologue: K^T, V, Q^T projections ----------
            # single PSUM pool for the WHOLE kernel: 4 x 2-bank "acc1024"
            # tiles shared by Q/K/V accumulators, score halves, ctx and out
            acc_scope = tc.tile_pool(name="acc_ps", bufs=4, space=PS)
            acc_psp = acc_scope.__enter__()
            with tc.tile_pool(name="xp", bufs=1) as xp:
                wkh = xp.tile([128, ET, O], BF16, tag="wkh")
                wkl = xp.tile([128, ET, O], BF16, tag="wkl")
                xkvh = xp.tile([128, ET, S], BF16, tag="xkvh")
                xkvl = xp.tile([128, ET, S], BF16, tag="xkvl")
                wqh = xp.tile([128, H, ET, O], BF16, tag="wqh")
                wql = xp.tile([128, H, ET, O], BF16, tag="wql")
                wvh = xp.tile([128, ET, O], BF16, tag="wvh")

                # DMA order = consumption order of the Q phase, striped over
                # the three issuing queues so the first matmul's operands
                # (wqh[h0] + xkvh e0 query-half) land within a few us:
                #   sync   : wqh h0, xkvh [0:SQ] by e, wkh, xkvh [SQ:S]
                #   scalar : wql h0, xkvl [0:SQ] by e, wkl+wvh, xkvl [SQ:S], wo
                #   gpsimd : wq heads 1..7 (consumed at ~10us/head)
                # head-0's operands land first so MM #1 starts ASAP; the
                # query-half x stream is striped over all three queues (the
                # Q phase is paced by its arrival), wq heads trickle between
                q3 = (nc.sync, nc.scalar, nc.gpsimd)
                # all of x-hi [0:SQ] first (both split3 passes 1-2 use it;
                # x-lo is only needed for the last pass per e-tile)
                nc.sync.dma_start(wqh[:, 0, 0:4, :], d_wqh[0, :, 0:512])
                nc.scalar.dma_start(wqh[:, 0, 4:8, :], d_wqh[0, :, 512:E])
                nc.scalar.dma_start(wql[:, 0, :, :], d_wql[0])
                nc.sync.dma_start(xkvh[:, 0, 0:512], d_xkvh[0:128, 0:512])
                nc.sync.dma_start(xkvh[:, 0, 512:SQ], d_xkvh[0:128, 512:SQ])
                nc.gpsimd.dma_start(xkvl[:, 0, 0:SQ], d_xkvl[0:128, 0:SQ])
                for e in range(1, ET):
                    q3[e % 3].dma_start(
                        xkvh[:, e, 0:SQ], d_xkvh[e * 128:(e + 1) * 128, 0:SQ])
                nc.gpsimd.dma_start(wqh[:, 1, :, :], d_wqh[1])
                nc.gpsimd.dma_start(wql[:, 1, :, :], d_wql[1])
                for e in range(1, ET):
                    q3[(e + 1) % 3].dma_start(
                        xkvl[:, e, 0:SQ], d_xkvl[e * 128:(e + 1) * 128, 0:SQ])
                for h in range(2, H):  # wq heads 2..7 after the xl stream
                    q3[h % 3].dma_start(wqh[:, h, :, :], d_wqh[h])
                    q3[h % 3].dma_start(wql[:, h, :, :], d_wql[h])
                nc.sync.dma_start(wkh[:], d_wkh)
                nc.scalar.dma_start(wkl[:], d_wkl)
                nc.scalar.dma_start(wvh[:], d_wvh)
                for e in range(ET):
                    q3[e % 3].dma_start(
                        xkvh[:, e, SQ:S], d_xkvh[e * 128:(e + 1) * 128, SQ:S])
                    q3[(e + 1) % 3].dma_start(
                        xkvl[:, e, SQ:S], d_xkvl[e * 128:(e + 1) * 128, SQ:S])
                nc.sync.dma_start(wo_sb[:], d_woh)

                # Q^T per head, then K^T, then V^T — ALL from the single
                # whole-kernel PSUM pool (acc_psp, opened outside), so no
                # pool-scope barrier ever stalls the PE between phases.
                for h in range(H):
                    q_ps = acc_psp.tile([128, SQ], F32, tag="acc1024")
                    for e in range(ET):
                        for ti, (w, xx) in enumerate(
                            ((wqh, xkvh), (wql, xkvh), (wqh, xkvl))
                        ):
                            for c in range(SQ // 512):
                                nc.tensor.matmul(
                                    q_ps[:, c * 512:(c + 1) * 512],
                                    w[:, h, e, :],
                                    xx[:, e, c * 512:(c + 1) * 512],
                                    start=(e == 0 and ti == 0),
                                    stop=(e == ET - 1 and ti == 2),
                                )
                    nc.scalar.copy(qth[:, h, :], q_ps[:])
                    nc.vector.tensor_sub(qtl[:, h, :], q_ps[:], qth[:, h, :])

                # K^T in two [128,1024] tiles; the first covers the query
                # half whose x is resident early.  Each 512-bank's hi/lo
                # copy is emitted as it completes.
                for t in range(2):
                    kt_ps = acc_psp.tile([128, SQ], F32, tag="acc1024")
                    for c in range(2):
                        g = t * 2 + c
                        gl = slice(g * 512, (g + 1) * 512)
                        cl = slice(c * 512, (c + 1) * 512)
                        for e in range(ET):
                            for ti, (w, xx) in enumerate(
                                ((wkh, xkvh), (wkl, xkvh), (wkh, xkvl))
                            ):
                                nc.tensor.matmul(
                                    kt_ps[:, cl],
                                    w[:, e, :],
                                    xx[:, e, gl],
                                    start=(e == 0 and ti == 0),
                                    stop=(e == ET - 1 and ti == 2),
                                )
                        nc.scalar.copy(kth[:, gl], kt_ps[:, cl])

                # V^T likewise, two tiles of two banks
                vt_sb = xp.tile([128, S], BF16, tag="vtsb")
                for t in range(2):
                    vt_ps = acc_psp.tile([128, SQ], F32, tag="acc1024")
                    for c in range(2):
                        g = t * 2 + c
                        gl = slice(g * 512, (g + 1) * 512)
                        cl = slice(c * 512, (c + 1) * 512)
                        for e in range(ET):
                            nc.tensor.matmul(
                                vt_ps[:, cl],
                                wvh[:, e, :],
                                xkvh[:, e, gl],
                                start=(e == 0),
                                stop=(e == ET - 1),
                            )
                        nc.scalar.copy(vt_sb[:, gl], vt_ps[:, cl])
                # V^T (o,s) -> V tiles (s-in-tile, st, o) on the DMA xbar
                nc.sync.dma_start_transpose(v_sb[:], vt_sb[:])

            # ---------------- main: per-head attention ------------------
            # PSUM budget (8 banks): "acc1024" 2-bank tiles x4 bufs shared by
            # score-halves, ctx and out accumulators.  Score halves cycle
            # through the free slots so the next q-tile's matmuls never wait
            # on this one's softmax.  P^T runs on the DMA xbar, not PE.
            with (
                tc.tile_pool(name="p_pool", bufs=8) as p_pool,
                tc.tile_pool(name="pt_pool", bufs=2) as pt_pool,
                tc.tile_pool(name="ctx_pool", bufs=H) as ctx_pool,
                tc.tile_pool(name="o_sb", bufs=2) as o_sbp,
            ):
                HS = S // 2  # 1024-wide score half

                MIN = mybir.AluOpType.min
                ctxs = []

                def emit_ctx_half(state, qc):
                    # ctx^T (o-part, q-free) accumulated over s-tiles; lagged
                    # into the next head's score phase as PE filler, one
                    # 512-wide half-burst at a time to limit the disruption.
                    # A fresh PSUM tile per half keeps the slot alive for
                    # only ~4us, so the score pipeline gets all four slots
                    # for most of the head.
                    pt_h = state["pt"]
                    ctx_h = state["ctx"]
                    ct_ps = acc_psp.tile([128, SQ], F32, tag="acc1024")
                    for st in range(ST):
                        nc.tensor.matmul(
                            ct_ps[:, 0:512],
                            v_sb[:, st, :],
                            pt_h[:, st, qc * 512:(qc + 1) * 512],
                            start=(st == 0),
                            stop=(st == ST - 1),
                        )
                    nc.scalar.copy(
                        ctx_h[:, qc * 512:(qc + 1) * 512], ct_ps[:, 0:512])

                pending_ctx = None
                for h in range(H):
                    pt_h = pt_pool.tile([128, ST, SQ], BF16, tag="pt")
                    for qt in range(QT):
                        # flash-style: each half gets a LOCAL max + exp so its
                        # PSUM slot frees without waiting for the other half;
                        # tiny per-partition factors fix up the normalization.
                        nm2 = tiny.tile([128, 2], F32, tag="nm2")
                        den2 = tiny.tile([128, 2], F32, tag="den2")
                        p_qt = p_pool.tile([128, S], BF16, tag="p")
                        for sh in range(2):
                            s_ps = acc_psp.tile([128, HS], F32, tag="acc1024")
                            for ti, qq in enumerate((qth, qtl)):
                                for c in range(2):
                                    nc.tensor.matmul(
                                        s_ps[:, c * 512:(c + 1) * 512],
                                        qq[:, h, qt * 128:(qt + 1) * 128],
                                        kth[:, sh * HS + c * 512:
                                            sh * HS + (c + 1) * 512],
                                        start=(ti == 0),
                                        stop=(ti == 1),
                                    )
                            # NOTE: per-bank maxes emitted mid-accumulation
                            # were tried and SLOWED the PE ~3% — DVE reads
                            # of a PSUM tile contend with the in-flight
                            # accumulation on the same bank pair
                            nc.vector.reduce_max(
                                out=nm2[:, sh:sh + 1], in_=s_ps[:],
                                axis=mybir.AxisListType.X, negate=True,
                            )
                            nc.scalar.activation(
                                p_qt[:, sh * HS:(sh + 1) * HS],
                                s_ps[:],
                                EXP, bias=nm2[:, sh:sh + 1], scale=1.0,
                                accum_out=den2[:, sh:sh + 1],
                            )
                        # fixup: p *= exp(m_sh - m_glob) / den_glob, all [128,·]
                        # tiny fixup chain on gpsimd — with 2-pass scores
                        # the DVE's two 1.13us maxes already fill its per-qt
                        # budget (these tiles are SBUF, so gpsimd is legal)
                        nmg = tiny.tile([128, 1], F32, tag="nmg")
                        nc.vector.tensor_reduce(
                            out=nmg[:], in_=nm2[:],
                            axis=mybir.AxisListType.X, op=MIN,
                        )
                        f2 = tiny.tile([128, 2], F32, tag="f2")
                        nc.scalar.activation(
                            f2[:], nm2[:], EXP, bias=nmg[:], scale=-1.0)
                        t2 = tiny.tile([128, 2], F32, tag="t2")
                        nc.gpsimd.tensor_mul(t2[:], den2[:], f2[:])
                        den = tiny.tile([128, 1], F32, tag="den")
                        nc.gpsimd.tensor_add(den[:], t2[:, 0:1], t2[:, 1:2])
                        invden = tiny.tile([128, 1], F32, tag="invden")
                        nc.vector.reciprocal(invden[:], den[:])
                        # renormalize the two halves on two engines
                        # concurrently — either engine alone is cadence-
                        # limiting.  (Keep the two-op tensor_scalar form:
                        # the single-op MULTIPLY,BYPASS lowering is ~10x
                        # slower on both engines.)
                        for sh, eng in ((0, nc.vector), (1, nc.gpsimd)):
                            eng.tensor_scalar(
                                out=p_qt[:, sh * HS:(sh + 1) * HS],
                                in0=p_qt[:, sh * HS:(sh + 1) * HS],
                                scalar1=f2[:, sh:sh + 1],
                                scalar2=invden[:],
                                op0=mybir.AluOpType.mult,
                                op1=mybir.AluOpType.mult,
                            )

                        # P^T on the DMA xbar: one [128,2048]->[128,16,128]
                        # transpose per q-tile.  DMA_TRANSPOSE occupies the
                        # issuing sequencer for the whole transfer, so keep
                        # them all on sync — the scalar queue must stay free
                        # for the latency-critical exp stream.
                        nc.sync.dma_start_transpose(
                            pt_h[:, :, qt * 128:(qt + 1) * 128], p_qt[:])

                        # ctx bursts at qt 3 and 7: the qt-7 burst sits on
                        # the head boundary, where the scalar engine's exp
                        # stream (2.9us/qt) otherwise falls behind the
                        # ctx-less PE cadence (2.6us/qt)
                        if pending_ctx is not None and qt in (3, 7):
                            emit_ctx_half(pending_ctx, qt // 4)
                            if qt == 7:
                                pending_ctx = None
                    ctx_h = ctx_pool.tile([128, SQ], BF16, tag="ctx")
                    pending_ctx = {"pt": pt_h, "ctx": ctx_h}
                    ctxs.append(ctx_h)

                # ------- out (q-part, e-free) = sum_h ctx_h^T-slices @ Wo_h
                # final head's ctx halves interleave with the out projection
                # so the PE tail stays dense
                def emit_out(qt):
                    # ec-outer: the first half's copy+store rides under the
                    # second half's matmuls.  Stores go on sync (hardware
                    # DGE) — gpsimd's software descriptor-gen costs ~650ns
                    # on the serial tail.
                    o_ps = acc_psp.tile([128, E], F32, tag="acc1024")
                    o_sb = o_sbp.tile([128, E], F32, tag="osb")
                    for ec in range(EC):
                        sl = slice(ec * 512, (ec + 1) * 512)
                        for h in range(H):
                            nc.tensor.matmul(
                                o_ps[:, sl],
                                ctxs[h][:, qt * 128:(qt + 1) * 128],
                                wo_sb[:, h, sl],
                                start=(h == 0),
                                stop=(h == H - 1),
                            )
                        cp = (nc.scalar.copy if (qt + ec) % 2
                              else nc.vector.tensor_copy)
                        cp(o_sb[:, sl], o_ps[:, sl])
                        dq = q3[(qt * EC + ec) % 3]
                        dq.dma_start(
                            d_out[qt * 128:(qt + 1) * 128, sl], o_sb[:, sl])

                emit_ctx_half(pending_ctx, 0)
                for qt in range(0, 4):
                    emit_out(qt)
                emit_ctx_half(pending_ctx, 1)
                for qt in range(4, QT - 1):
                    emit_out(qt)
                # last row-block: quarter-stores over three queues so the
                # final transfer off-chip is ~1us, not ~2.5us
                qt = QT - 1
                o_ps = acc_psp.tile([128, E], F32, tag="acc1024")
                o_sb = o_sbp.tile([128, E], F32, tag="osb")
                for ec in range(EC):
                    sl = slice(ec * 512, (ec + 1) * 512)
                    for h in range(H):
                        nc.tensor.matmul(
                            o_ps[:, sl],
                            ctxs[h][:, qt * 128:(qt + 1) * 128],
                            wo_sb[:, h, sl],
                            start=(h == 0),
                            stop=(h == H - 1),
                        )
                    cp = (nc.scalar.copy if ec else nc.vector.tensor_copy)
                    cp(o_sb[:, sl], o_ps[:, sl])
                    for qq in range(2):
                        qsl = slice(ec * 512 + qq * 256,
                                    ec * 512 + (qq + 1) * 256)
                        q3[(ec * 2 + qq) % 3].dma_start(
                            d_out[qt * 128:(qt + 1) * 128, qsl], o_sb[:, qsl])
            acc_scope.__exit__(None, None, None)

    nc.compile()
    return nc


def _split(a):
    """fp32 -> (hi, lo) bf16 pair with hi + lo ~= a."""
    hi = a.astype(ml_dtypes.bfloat16)
    lo = (a - hi.astype(np.float32)).astype(ml_dtypes.bfloat16)
    return hi, lo


def _tile_rows(a, inner):
    """(T*128, inner) -> (128, T*inner): row p holds tiles' p-th rows."""
    t = a.shape[0] // 128
    return np.ascontiguousarray(
        a.reshape(t, 128, inner).transpose(1, 0, 2).reshape(128, t * inner))


def _prepare_inputs(x, Wq, Wk, Wv, Wo):
    x = np.asarray(x, dtype=np.float32)
    scale = np.float32(1.0 / np.sqrt(O))
    wqh, wql = _split(np.asarray(Wq, dtype=np.float32) * scale)
    wkh, wkl = _split(np.asarray(Wk, dtype=np.float32))
    wvh = np.asarray(Wv, dtype=np.float32).astype(ml_dtypes.bfloat16)
    woh = np.asarray(Wo, dtype=np.float32).astype(ml_dtypes.bfloat16)

    # re-layout so every device DMA row is a contiguous 2KB+ run
    wqh = np.stack([_tile_rows(wqh[h], O) for h in range(H)])  # (H,128,E)
    wql = np.stack([_tile_rows(wql[h], O) for h in range(H)])
    wkh = _tile_rows(wkh, O)                                   # (128,E)
    wkl = _tile_rows(wkl, O)
    wvh = _tile_rows(wvh, O)
    woh = _tile_rows(woh, E)                                   # (128,H*E)

    in_maps = []
    xsplits = {}
    for b in range(B):
        xsplits[b] = _split(np.ascontiguousarray(x[b].T))  # (E, S) fp32
    for c in range(NCORES):
        b, half = divmod(c, 2)
        xh, xl = xsplits[b]
        if half == 0:
            ph, pl = xh, xl
        else:
            # rotate so this core's query half occupies columns [0, SQ);
            # attention is permutation-invariant over the key/value axis
            ph = np.ascontiguousarray(np.roll(xh, SQ, axis=1))
            pl = np.ascontiguousarray(np.roll(xl, SQ, axis=1))
        in_maps.append({
            "xkvh": ph, "xkvl": pl,
            "wqh": wqh, "wql": wql,
            "wkh": wkh, "wkl": wkl, "wvh": wvh, "woh": woh,
        })
    return in_maps


def kernel(x, Wq, Wk, Wv, Wo):
    global _compiled
    from concourse.bass_utils import run_bass_kernel_spmd

    x = np.asarray(x, dtype=np.float32)

    if _compiled is None:
        _compiled = _build()
    nc = _compiled

    in_maps = _prepare_inputs(x, Wq, Wk, Wv, Wo)

    res = None
    for attempt in range(3):
        try:
            res = run_bass_kernel_spmd(
                nc, in_maps, core_ids=list(range(NCORES)))
            break
        except Exception:
            # transient device wedge — retry; re-raise on the last attempt
            if attempt == 2:
                raise

    out = np.empty((B, S, E), dtype=np.float32)
    for c in range(NCORES):
        b, half = divmod(c, 2)
        out[b, half * SQ:(half + 1) * SQ, :] = res.results[c]["out"]
    return out
